# revision 5
# baseline (speedup 1.0000x reference)
"""Trainium2 Bass kernel: MinEntropyConsensusLoss.

Reference computation:
    lx = log_softmax(x, axis=1); ly = log_softmax(y, axis=1)
    ce = 0.5 * (-(lx + ly)).min(axis=1)          # [N]
    out = ce.mean()                               # scalar

Identity used here:
    -(lx + ly)[n, c] = lse_x[n] + lse_y[n] - (x + y)[n, c]
    min_c(...)       = lse_x[n] + lse_y[n] - max_c(x + y)[n]
so per row only three free-dim reductions are needed:
    sum(exp(x)) and sum(exp(y))   -> ACT engine, exp with accumulate
    max(x + y)                    -> DVE tensor_tensor_reduce (add+max fused)
Inputs are N(0,1) so unshifted exp() stays comfortably inside f32 range.

Sharding: data-parallel on N across the 8 NeuronCores (4096 rows each).
Each core emits a [128, 1] vector of per-partition partial sums of
(ln sx + ln sy - max(x+y)); the host finishes the mean.
"""

import numpy as np

N, C = 32768, 2048
NCORES = 8
NPER = N // NCORES  # 4096 rows per core
P = 128             # SBUF partitions
Q = 2               # 128-row blocks loaded per DMA (2 MB transfers)
NBLK = NPER // P    # 32 row-blocks per core
NITER = NBLK // Q   # 16 unrolled iterations

_cache: dict = {}


def _split_waits(nc, max_waits=1):
    """This container's pinned walrus encodes at most one sync-wait per
    instruction; hoist extra waits onto preceding NoOps (same engine, so
    wait-for-all semantics are preserved)."""
    from concourse import mybir

    for f in nc.m.functions:
        for blk in f.blocks:
            i = 0
            insts = blk.instructions
            while i < len(insts):
                inst = insts[i]
                si = getattr(inst, "sync_info", None)
                if si is not None and si.on_wait and len(si.on_wait) > max_waits:
                    waits = list(si.on_wait)
                    head, tail = waits[:-max_waits], waits[-max_waits:]
                    pos = i
                    for k in range(0, len(head), max_waits):
                        nop = mybir.InstNoOp(
                            name=nc.get_next_instruction_name(),
                            ins=[], outs=[],
                            engine=inst.engine,
                            sync_info=mybir.SyncInfo(
                                on_wait=head[k : k + max_waits], on_update=[]
                            ),
                        )
                        insts.insert(pos, nop)
                        pos += 1
                        i += 1
                    inst.sync_info = mybir.SyncInfo(
                        on_wait=tail, on_update=list(si.on_update)
                    )
                i += 1


def _build_nc(reps=1):
    """reps>1 repeats the whole computation back-to-back (one output column
    per rep) — used only by the timing harness to measure steady-state
    per-exec time as a slope within a single NEFF execution."""
    import concourse.bacc as bacc
    import concourse.tile as tile
    from concourse import mybir

    f32 = mybir.dt.float32
    AF = mybir.ActivationFunctionType

    nc = bacc.Bacc("TRN2", num_devices=NCORES)
    x = nc.dram_tensor("x", [NPER, C], f32, kind="ExternalInput")
    y = nc.dram_tensor("y", [NPER, C], f32, kind="ExternalInput")
    out = nc.dram_tensor("part", [P, reps], f32, kind="ExternalOutput")

    # row n = (i*Q + q)*P + p  ->  block b = i*Q + q on partition p
    xb = x.ap().rearrange("(i q p) c -> i p q c", p=P, q=Q)
    yb = y.ap().rearrange("(i q p) c -> i p q c", p=P, q=Q)

    with tile.TileContext(nc) as tc:
        with (
            tc.tile_pool(name="io", bufs=3) as io,
            tc.tile_pool(name="sc", bufs=2) as scp,
            tc.tile_pool(name="accp", bufs=2) as accp,
        ):
            for rep in range(reps):
                sx_acc = accp.tile([P, NBLK], f32, tag="sx")
                sy_acc = accp.tile([P, NBLK], f32, tag="sy")
                mxy_acc = accp.tile([P, NBLK], f32, tag="mxy")

                for i in range(NITER):
                    x_t = io.tile([P, Q, C], f32, tag="x")
                    nc.sync.dma_start(out=x_t, in_=xb[i, :, :, :])
                    y_t = io.tile([P, Q, C], f32, tag="y")
                    nc.sync.dma_start(out=y_t, in_=yb[i, :, :, :])
                    for q in range(Q):
                        b = i * Q + q
                        ex = scp.tile([P, C], f32, tag="ex")
                        nc.scalar.activation(
                            out=ex, in_=x_t[:, q, :], func=AF.Exp,
                            accum_out=sx_acc[:, b : b + 1],
                        )
                        ey = scp.tile([P, C], f32, tag="ey")
                        nc.scalar.activation(
                            out=ey, in_=y_t[:, q, :], func=AF.Exp,
                            accum_out=sy_acc[:, b : b + 1],
                        )
                        s = scp.tile([P, C], f32, tag="s")
                        nc.vector.tensor_add(s, x_t[:, q, :], y_t[:, q, :])
                        nc.vector.reduce_max(
                            out=mxy_acc[:, b : b + 1], in_=s,
                            axis=mybir.AxisListType.X,
                        )

                # epilogue: part[p] = sum_b (ln sx + ln sy - mxy)[p, b]
                lsx = accp.tile([P, NBLK], f32, tag="lsx")
                lsy = accp.tile([P, NBLK], f32, tag="lsy")
                nc.scalar.activation(out=lsx, in_=sx_acc, func=AF.Ln)
                nc.scalar.activation(out=lsy, in_=sy_acc, func=AF.Ln)
                lsum = accp.tile([P, NBLK], f32, tag="lsum")
                nc.vector.tensor_add(lsum, lsx, lsy)
                u = accp.tile([P, NBLK], f32, tag="u")
                nc.vector.tensor_sub(u, lsum, mxy_acc)
                part = accp.tile([P, 1], f32, tag="part")
                nc.vector.reduce_sum(out=part, in_=u, axis=mybir.AxisListType.X)
                nc.sync.dma_start(out=out.ap()[:, rep : rep + 1], in_=part)
    nc.compile()
    _split_waits(nc)
    return nc


def _get_nc():
    if "nc" not in _cache:
        _cache["nc"] = _build_nc()
    return _cache["nc"]


def _make_in_maps(x: np.ndarray, y: np.ndarray):
    in_maps = []
    for k in range(NCORES):
        sl = slice(k * NPER, (k + 1) * NPER)
        in_maps.append({"x": x[sl], "y": y[sl]})
    return in_maps


def kernel(x, y):
    import concourse.bass_utils as bass_utils

    x = np.ascontiguousarray(np.asarray(x, dtype=np.float32))
    y = np.ascontiguousarray(np.asarray(y, dtype=np.float32))
    assert x.shape == (N, C) and y.shape == (N, C)

    nc = _get_nc()
    res = bass_utils.run_bass_kernel_spmd(
        nc, _make_in_maps(x, y), core_ids=list(range(NCORES))
    )
    total = sum(float(r["part"].sum(dtype=np.float64)) for r in res.results)
    return np.float32(0.5 * total / N)


if __name__ == "__main__":
    rng = np.random.default_rng(0)
    x = rng.standard_normal((N, C), dtype=np.float32)
    y = rng.standard_normal((N, C), dtype=np.float32)
    got = kernel(x=x, y=y)
    lx = x - np.log(np.exp(x).sum(1, keepdims=True))
    ly = y - np.log(np.exp(y).sum(1, keepdims=True))
    want = (0.5 * (-(lx + ly)).min(1)).mean()
    print("kernel:", got, "numpy:", want, "rel err:", abs(got - want) / abs(want))
